# revision 1
# baseline (speedup 1.0000x reference)
import sys

sys.path.insert(0, "/opt/trn_rl_repo")
import numpy as np

import concourse.bacc as bacc
import concourse.mybir as mybir
from concourse.bass_utils import run_bass_kernel_spmd
from concourse.tile import TileContext

FP32 = mybir.dt.float32
FP32R = mybir.dt.float32r
AF = mybir.ActivationFunctionType
ALU = mybir.AluOpType

NCORES = 8
B = 262144
BC = B // NCORES  # 32768 points per core
NT = 512  # points per tile (free dim)
NTILES = BC // NT  # 64
L_XYZ, L_DIR, WIDTH = 10, 4, 256
TWO_PI = 6.283185307179586
INV_2PI = 1.0 / TWO_PI
MAGIC = 12582912.0  # 1.5 * 2**23, round-to-nearest-int trick

# ---------------- weight-pack column layout ----------------
_OFFS = {}
_NCOLS = 0


def _build_offsets():
    global _NCOLS
    col = 0

    def add(key, m):
        nonlocal col
        _OFFS[key] = col
        col += m

    for m in range(2):
        add(("l1", 0, m), 128)
    for name in ("l2", "l3", "l4", "l6", "l7", "l8", "lc"):
        for k in range(2):
            for m in range(2):
                add((name, k, m), 128)
    for k in range(3):
        for m in range(2):
            add(("l5", k, m), 128)
    add(("ws", 0, 0), 1)
    add(("ws", 1, 0), 1)
    for k in range(3):
        add(("wd", k, 0), 128)
    add(("wo", 0, 0), 3)
    _NCOLS = col


_build_offsets()  # _NCOLS == 4997

_CACHE = {}


def _build():
    if "nc" in _CACHE:
        return _CACHE["nc"]
    nc = bacc.Bacc("TRN2", target_bir_lowering=False)

    xrep_ext = nc.declare_dram_parameter("xrep", (60, BC), FP32, isOutput=False)
    d0_ext = nc.declare_dram_parameter("d0", (128, 768), FP32, isOutput=False)
    wp_ext = nc.declare_dram_parameter("wp", (128, _NCOLS), FP32, isOutput=False)
    bp_ext = nc.declare_dram_parameter("bp", (128, 21), FP32, isOutput=False)
    cx_ext = nc.declare_dram_parameter("cx", (60, 2), FP32, isOutput=False)
    cd_ext = nc.declare_dram_parameter("cd", (24, 2), FP32, isOutput=False)
    drep_ext = nc.declare_dram_parameter("drep", (24, BC), FP32, isOutput=True)
    rgbT_ext = nc.declare_dram_parameter("rgbT", (3, BC), FP32, isOutput=True)
    sigT_ext = nc.declare_dram_parameter("sigT", (1, BC), FP32, isOutput=True)

    with TileContext(nc) as tc:
        with (
            tc.tile_pool(name="cp", bufs=1) as cp,
            tc.tile_pool(name="wk", bufs=2) as wk,
            tc.tile_pool(name="ps", bufs=4, space="PSUM") as ps,
            tc.tile_pool(name="ps2", bufs=2, space="PSUM") as ps2,
        ):
            WP = cp.tile([128, _NCOLS], FP32R)
            nc.sync.dma_start(out=WP[:], in_=wp_ext[:].bitcast(FP32R))
            BP = cp.tile([128, 21], FP32)
            nc.sync.dma_start(out=BP[:], in_=bp_ext[:])
            CX = cp.tile([60, 2], FP32)
            nc.sync.dma_start(out=CX[:], in_=cx_ext[:])
            CD = cp.tile([24, 2], FP32)
            nc.sync.dma_start(out=CD[:], in_=cd_ext[:])

            # ---- d normalization: d0[p, c*256+t] = d[p*256+t, c] ----
            D0 = cp.tile([128, 768], FP32)
            nc.sync.dma_start(out=D0[:], in_=d0_ext[:])
            SQ = cp.tile([128, 768], FP32)
            nc.vector.tensor_tensor(out=SQ[:], in0=D0[:], in1=D0[:], op=ALU.mult)
            S = cp.tile([128, 256], FP32)
            nc.vector.tensor_tensor(
                out=S[:], in0=SQ[:, 0:256], in1=SQ[:, 256:512], op=ALU.add
            )
            nc.vector.tensor_tensor(
                out=S[:], in0=S[:], in1=SQ[:, 512:768], op=ALU.add
            )
            NRM = cp.tile([128, 256], FP32)
            nc.scalar.activation(out=NRM[:], in_=S[:], func=AF.Sqrt)
            nc.vector.tensor_scalar(
                out=NRM[:], in0=NRM[:], scalar1=1e-8, scalar2=None, op0=ALU.add
            )
            R = cp.tile([128, 256], FP32)
            nc.vector.reciprocal(out=R[:], in_=NRM[:])
            U = cp.tile([128, 768], FP32)
            for c in range(3):
                nc.vector.tensor_tensor(
                    out=U[:, 256 * c : 256 * (c + 1)],
                    in0=D0[:, 256 * c : 256 * (c + 1)],
                    in1=R[:],
                    op=ALU.mult,
                )
            # transpose-write unit dirs to DRAM, replicated 8x:
            # drep[3g+c, p*256+j] = U[p, c*256+j]
            for g in range(8):
                nc.sync.dma_start(
                    out=drep_ext[3 * g : 3 * g + 3, :].rearrange(
                        "c (p j) -> p c j", p=128
                    ),
                    in_=U[:].rearrange("p (c j) -> p c j", c=3),
                )

            def enc(src_ext, consts, rows, out_rows, tagp):
                # load raw coords (rows, NT) and produce fp32r PE tile
                # [sin/cos rows, raw coords rows]
                XS = wk.tile([rows, NT], FP32, tag=tagp + "xs")
                nc.sync.dma_start(out=XS[:], in_=src_ext)
                Y = wk.tile([rows, NT], FP32, tag=tagp + "y")
                nc.vector.tensor_scalar(
                    out=Y[:], in0=XS[:], scalar1=consts[:, 0:1],
                    scalar2=consts[:, 1:2], op0=ALU.mult, op1=ALU.add,
                )
                K1 = wk.tile([rows, NT], FP32, tag=tagp + "k1")
                nc.vector.tensor_scalar(
                    out=K1[:], in0=Y[:], scalar1=MAGIC, scalar2=None, op0=ALU.add
                )
                K2 = wk.tile([rows, NT], FP32, tag=tagp + "k2")
                nc.vector.tensor_scalar(
                    out=K2[:], in0=K1[:], scalar1=-MAGIC, scalar2=None, op0=ALU.add
                )
                F = wk.tile([rows, NT], FP32, tag=tagp + "f")
                nc.vector.tensor_tensor(out=F[:], in0=Y[:], in1=K2[:], op=ALU.subtract)
                PE = wk.tile([out_rows, NT], FP32R, tag=tagp + "pe")
                nc.scalar.activation(
                    out=PE[0:rows, :], in_=F[:], func=AF.Sin, scale=TWO_PI
                )
                return PE

            def evac_pair(plo, phi, bq, tagp):
                olo = wk.tile([128, NT], FP32R, tag=tagp + "lo")
                ohi = wk.tile([128, NT], FP32R, tag=tagp + "hi")
                nc.scalar.activation(
                    out=olo[:], in_=plo[:], func=AF.Relu,
                    bias=BP[:, 2 * bq : 2 * bq + 1],
                )
                nc.vector.tensor_scalar(
                    out=ohi[:], in0=phi[:], scalar1=BP[:, 2 * bq + 1 : 2 * bq + 2],
                    scalar2=0.0, op0=ALU.add, op1=ALU.max,
                )
                return olo, ohi

            def mid_layer(name, rlo, rhi, bq):
                plo = ps.tile([128, NT], FP32, tag="pm")
                phi = ps.tile([128, NT], FP32, tag="pm")
                for k, r in enumerate((rlo, rhi)):
                    o0 = _OFFS[(name, k, 0)]
                    o1 = _OFFS[(name, k, 1)]
                    nc.tensor.matmul(
                        plo[:], WP[0:128, o0 : o0 + 128], r,
                        start=(k == 0), stop=(k == 1),
                    )
                    nc.tensor.matmul(
                        phi[:], WP[0:128, o1 : o1 + 128], r,
                        start=(k == 0), stop=(k == 1),
                    )
                return evac_pair(plo, phi, bq, name)

            for j in range(NTILES):
                j0, j1 = NT * j, NT * (j + 1)
                PEX = enc(xrep_ext[:, j0:j1], CX, 60, 63, "x")
                nc.sync.dma_start(
                    out=PEX[60:63, :], in_=xrep_ext[0:3, j0:j1].bitcast(FP32R)
                )
                PED = enc(drep_ext[:, j0:j1], CD, 24, 27, "d")
                nc.sync.dma_start(
                    out=PED[24:27, :], in_=drep_ext[0:3, j0:j1].bitcast(FP32R)
                )

                # layer 1: K=63 (PE), M=256
                p1lo = ps.tile([128, NT], FP32, tag="pm")
                p1hi = ps.tile([128, NT], FP32, tag="pm")
                o0 = _OFFS[("l1", 0, 0)]
                o1 = _OFFS[("l1", 0, 1)]
                nc.tensor.matmul(
                    p1lo[:], WP[0:63, o0 : o0 + 128], PEX[0:63, :],
                    start=True, stop=True,
                )
                nc.tensor.matmul(
                    p1hi[:], WP[0:63, o1 : o1 + 128], PEX[0:63, :],
                    start=True, stop=True,
                )
                h1lo, h1hi = evac_pair(p1lo, p1hi, 0, "h1")

                h2lo, h2hi = mid_layer("l2", h1lo[:], h1hi[:], 1)
                h3lo, h3hi = mid_layer("l3", h2lo[:], h2hi[:], 2)
                h4lo, h4hi = mid_layer("l4", h3lo[:], h3hi[:], 3)

                # layer 5: K = 256 (h4) + 63 (PE skip)
                p5lo = ps.tile([128, NT], FP32, tag="pm")
                p5hi = ps.tile([128, NT], FP32, tag="pm")
                rhss = (h4lo[:], h4hi[:], PEX[0:63, :])
                kk = (128, 128, 63)
                for k in range(3):
                    o0 = _OFFS[("l5", k, 0)]
                    o1 = _OFFS[("l5", k, 1)]
                    nc.tensor.matmul(
                        p5lo[:], WP[0 : kk[k], o0 : o0 + 128], rhss[k],
                        start=(k == 0), stop=(k == 2),
                    )
                    nc.tensor.matmul(
                        p5hi[:], WP[0 : kk[k], o1 : o1 + 128], rhss[k],
                        start=(k == 0), stop=(k == 2),
                    )
                h5lo, h5hi = evac_pair(p5lo, p5hi, 4, "h5")

                h6lo, h6hi = mid_layer("l6", h5lo[:], h5hi[:], 5)
                h7lo, h7hi = mid_layer("l7", h6lo[:], h6hi[:], 6)
                h8lo, h8hi = mid_layer("l8", h7lo[:], h7hi[:], 7)

                # sigma head: (1,256) @ h8
                psg = ps2.tile([1, NT], FP32, tag="psg")
                ow0 = _OFFS[("ws", 0, 0)]
                ow1 = _OFFS[("ws", 1, 0)]
                nc.tensor.matmul(
                    psg[:], WP[0:128, ow0 : ow0 + 1], h8lo[:],
                    start=True, stop=False,
                )
                nc.tensor.matmul(
                    psg[:], WP[0:128, ow1 : ow1 + 1], h8hi[:],
                    start=False, stop=True,
                )
                SIG = wk.tile([1, NT], FP32, tag="sig")
                nc.scalar.activation(
                    out=SIG[:], in_=psg[:], func=AF.Relu, bias=BP[0:1, 19:20]
                )
                nc.sync.dma_start(out=sigT_ext[:, j0:j1], in_=SIG[:])

                # color branch
                hclo, hchi = mid_layer("lc", h8lo[:], h8hi[:], 8)

                # Wd: K = 256 (hc) + 27 (PE dir)
                phd = ps.tile([128, NT], FP32, tag="pm")
                rhss = (hclo[:], hchi[:], PED[0:27, :])
                kk = (128, 128, 27)
                for k in range(3):
                    od = _OFFS[("wd", k, 0)]
                    nc.tensor.matmul(
                        phd[:], WP[0 : kk[k], od : od + 128], rhss[k],
                        start=(k == 0), stop=(k == 2),
                    )
                HD = wk.tile([128, NT], FP32R, tag="hd")
                nc.vector.tensor_scalar(
                    out=HD[:], in0=phd[:], scalar1=BP[:, 18:19], scalar2=0.0,
                    op0=ALU.add, op1=ALU.max,
                )

                # Wo + sigmoid via 0.5*tanh(0.5 z)+0.5
                po = ps2.tile([3, NT], FP32, tag="po")
                oo = _OFFS[("wo", 0, 0)]
                nc.tensor.matmul(
                    po[:], WP[0:128, oo : oo + 3], HD[:], start=True, stop=True
                )
                T3 = wk.tile([3, NT], FP32, tag="t3")
                nc.scalar.activation(
                    out=T3[:], in_=po[:], func=AF.Tanh, bias=BP[0:3, 20:21],
                    scale=0.5,
                )
                RGB = wk.tile([3, NT], FP32, tag="rgb")
                nc.vector.tensor_scalar(
                    out=RGB[:], in0=T3[:], scalar1=0.5, scalar2=0.5,
                    op0=ALU.mult, op1=ALU.add,
                )
                nc.sync.dma_start(out=rgbT_ext[:, j0:j1], in_=RGB[:])

    nc.compile()
    _CACHE["nc"] = nc
    return nc


def _enc_consts(L):
    out = np.zeros((6 * L, 2), dtype=np.float32)
    for i in range(L):
        for c in range(6):
            out[6 * i + c, 0] = (2.0**i) * INV_2PI
            out[6 * i + c, 1] = 0.0 if c < 3 else 0.25
    return out


def _pack_weights(inp):
    wp = np.zeros((128, _NCOLS), dtype=np.float32)

    def put(key, block):
        off = _OFFS[key]
        K, M = block.shape
        wp[0:K, off : off + M] = block

    W1 = np.asarray(inp["W1"], np.float32)
    l1T = np.concatenate([W1[:, 3:], W1[:, :3]], axis=1).T  # (63, 256)
    put(("l1", 0, 0), l1T[:, 0:128])
    put(("l1", 0, 1), l1T[:, 128:256])
    for name, wname in (
        ("l2", "W2"), ("l3", "W3"), ("l4", "W4"),
        ("l6", "W6"), ("l7", "W7"), ("l8", "W8"), ("lc", "Wc"),
    ):
        WT = np.asarray(inp[wname], np.float32).T  # (256, 256)
        for k in range(2):
            for m in range(2):
                put((name, k, m),
                    WT[128 * k : 128 * (k + 1), 128 * m : 128 * (m + 1)])
    W5 = np.asarray(inp["W5"], np.float32)  # (256, 319); cols [h(256), x(3), enc(60)]
    W5T = np.concatenate([W5[:, :256], W5[:, 259:319], W5[:, 256:259]], axis=1).T
    for k, (a, b) in enumerate(((0, 128), (128, 256), (256, 319))):
        for m in range(2):
            put(("l5", k, m), W5T[a:b, 128 * m : 128 * (m + 1)])
    WsT = np.asarray(inp["Ws"], np.float32).T  # (256, 1)
    put(("ws", 0, 0), WsT[0:128])
    put(("ws", 1, 0), WsT[128:256])
    Wd = np.asarray(inp["Wd"], np.float32)  # (128, 283); cols [hc(256), d(3), enc(24)]
    WdT = np.concatenate([Wd[:, :256], Wd[:, 259:283], Wd[:, 256:259]], axis=1).T
    for k, (a, b) in enumerate(((0, 128), (128, 256), (256, 283))):
        put(("wd", k, 0), WdT[a:b, :])
    put(("wo", 0, 0), np.asarray(inp["Wo"], np.float32).T)  # (128, 3)
    return wp


def _pack_biases(inp):
    bp = np.zeros((128, 21), dtype=np.float32)
    for q, nm in enumerate(("b1", "b2", "b3", "b4", "b5", "b6", "b7", "b8", "bc")):
        b = np.asarray(inp[nm], np.float32)
        bp[:, 2 * q] = b[0:128]
        bp[:, 2 * q + 1] = b[128:256]
    bp[:, 18] = np.asarray(inp["bd"], np.float32)
    bp[0, 19] = float(np.asarray(inp["bs"], np.float32).reshape(-1)[0])
    bp[0:3, 20] = 0.5 * np.asarray(inp["bo"], np.float32)
    return bp


def kernel(**inputs):
    nc = _build()
    x = np.ascontiguousarray(np.asarray(inputs["x_world"], np.float32))
    d = np.ascontiguousarray(np.asarray(inputs["d"], np.float32))
    wp = _pack_weights(inputs)
    bp = _pack_biases(inputs)
    cx = _enc_consts(L_XYZ)
    cd = _enc_consts(L_DIR)

    in_maps = []
    for c in range(NCORES):
        sl = slice(c * BC, (c + 1) * BC)
        xrep = np.ascontiguousarray(np.tile(x[sl].T, (20, 1)))  # (60, BC)
        d0 = np.ascontiguousarray(
            d[sl].reshape(128, 256, 3).transpose(0, 2, 1).reshape(128, 768)
        )
        in_maps.append(
            {"xrep": xrep, "d0": d0, "wp": wp, "bp": bp, "cx": cx, "cd": cd}
        )
    res = run_bass_kernel_spmd(nc, in_maps, list(range(NCORES)))
    rgb = np.concatenate(
        [res.results[c]["rgbT"].T for c in range(NCORES)], axis=0
    ).astype(np.float32)
    sigma = np.concatenate(
        [res.results[c]["sigT"].T for c in range(NCORES)], axis=0
    ).astype(np.float32)
    return rgb, sigma


# revision 20
# speedup vs baseline: 6515.4789x; 6515.4789x over previous
import sys

sys.path.insert(0, "/opt/trn_rl_repo")
import numpy as np

import concourse.bacc as bacc
import concourse.mybir as mybir
from concourse.bass_utils import run_bass_kernel_spmd
from concourse.hw_specs import get_activation_tables as _gat
from concourse.tile import TileContext


def _gat_patched(arch):
    # Restrict the act-table load pass to sets that jointly serve all the
    # functions we use (silu_and_others: sin+tanh+relu), so no per-tile
    # table swaps are inserted. Set ids stay aligned with act_info.json.
    full = _gat(arch)
    keep = {"silu_and_others", "sqrt_and_others"}
    return {n: (s if n in keep else set()) for n, s in full.items()}


bacc.get_activation_tables = _gat_patched

FP32 = mybir.dt.float32
FP32R = mybir.dt.float32r
AF = mybir.ActivationFunctionType
ALU = mybir.AluOpType

NCORES = 8
B = 262144
BC = B // NCORES  # 32768 points per core
NT = 512  # points per tile (free dim)
NTILES = BC // NT  # 64
L_XYZ, L_DIR, WIDTH = 10, 4, 256
TWO_PI = 6.283185307179586
INV_2PI = 1.0 / TWO_PI
MAGIC = 12582912.0  # 1.5 * 2**23, round-to-nearest-int trick

# ---------------- weight-pack column layout ----------------
_OFFS = {}
_NCOLS = 0


def _build_offsets():
    global _NCOLS
    col = 0

    def add(key, m):
        nonlocal col
        _OFFS[key] = col
        col += m

    for m in range(2):
        add(("l1", 0, m), 128)
    for name in ("l2", "l3", "l4", "l6", "l7", "l8", "lc"):
        for k in range(2):
            for m in range(2):
                add((name, k, m), 128)
    for k in range(3):
        for m in range(2):
            add(("l5", k, m), 128)
    add(("ws", 0, 0), 1)
    add(("ws", 1, 0), 1)
    for k in range(3):
        add(("wd", k, 0), 128)
    add(("wo", 0, 0), 3)
    _NCOLS = col


_build_offsets()  # _NCOLS == 4997

_CACHE = {}


def _build():
    if "nc" in _CACHE:
        return _CACHE["nc"]
    nc = bacc.Bacc("TRN2", target_bir_lowering=False)

    xrep_ext = nc.declare_dram_parameter("xrep", (64, BC), FP32, isOutput=False)
    d0_ext = nc.declare_dram_parameter("d0", (128, 768), FP32, isOutput=False)
    wp_ext = nc.declare_dram_parameter("wp", (128, _NCOLS), FP32, isOutput=False)
    bp_ext = nc.declare_dram_parameter("bp", (128, 21), FP32, isOutput=False)
    cxd_ext = nc.declare_dram_parameter("cxd", (88, 2), FP32, isOutput=False)
    # drep split in two so early tiles don't wait on the full transpose
    NHEAD = 8  # tiles covered by the head chunk
    drh_ext = nc.declare_dram_parameter(
        "drh", (24, NT * NHEAD), FP32, isOutput=True
    )
    drt_ext = nc.declare_dram_parameter(
        "drt", (24, BC - NT * NHEAD), FP32, isOutput=True
    )
    rgbT_ext = nc.declare_dram_parameter("rgbT", (3, BC), FP32, isOutput=True)
    sigT_ext = nc.declare_dram_parameter("sigT", (1, BC), FP32, isOutput=True)

    with TileContext(nc) as tc:
        with (
            tc.tile_pool(name="cp", bufs=1) as cp,
            tc.tile_pool(name="wk", bufs=2) as wk,
            tc.tile_pool(name="pp", bufs=4) as pp,
            tc.tile_pool(name="ps", bufs=6, space="PSUM") as ps,
            tc.tile_pool(name="ps2", bufs=2, space="PSUM") as ps2,
        ):
            # ---- d normalization: d0[p, c*256+t] = d[p*256+t, c] ----
            # D0 first: it heads the startup critical path
            D0 = cp.tile([128, 768], FP32)
            nc.sync.dma_start(out=D0[:], in_=d0_ext[:])
            WP = cp.tile([128, _NCOLS], FP32R)
            nc.sync.dma_start(out=WP[:], in_=wp_ext[:].bitcast(FP32R))
            BP = cp.tile([128, 21], FP32)
            nc.sync.dma_start(out=BP[:], in_=bp_ext[:])
            CXD = cp.tile([88, 2], FP32)
            nc.sync.dma_start(out=CXD[:], in_=cxd_ext[:])
            SQ = cp.tile([128, 768], FP32)
            nc.vector.tensor_tensor(out=SQ[:], in0=D0[:], in1=D0[:], op=ALU.mult)
            S = cp.tile([128, 256], FP32)
            nc.vector.tensor_tensor(
                out=S[:], in0=SQ[:, 0:256], in1=SQ[:, 256:512], op=ALU.add
            )
            nc.vector.tensor_tensor(
                out=S[:], in0=S[:], in1=SQ[:, 512:768], op=ALU.add
            )
            NRM = cp.tile([128, 256], FP32)
            nc.scalar.activation(out=NRM[:], in_=S[:], func=AF.Sqrt)
            nc.vector.tensor_scalar(
                out=NRM[:], in0=NRM[:], scalar1=1e-8, scalar2=None, op0=ALU.add
            )
            R = cp.tile([128, 256], FP32)
            nc.vector.reciprocal(out=R[:], in_=NRM[:])
            U = cp.tile([128, 768], FP32)
            for c in range(3):
                nc.vector.tensor_tensor(
                    out=U[:, 256 * c : 256 * (c + 1)],
                    in0=D0[:, 256 * c : 256 * (c + 1)],
                    in1=R[:],
                    op=ALU.mult,
                )
            # transpose-write unit dirs to DRAM, replicated 8x:
            # drep[3g+c, p*256+j] = U[p, c*256+j]
            # head tensor (first NHEAD tiles) first so the tile pipeline
            # can start before the full transpose completes
            PH = 2 * NHEAD  # partitions covered by head
            for g in range(8):
                nc.sync.dma_start(
                    out=drh_ext[3 * g : 3 * g + 3, :].rearrange(
                        "c (p j) -> p c j", p=PH
                    ),
                    in_=U[0:PH, :].rearrange("p (c j) -> p c j", c=3),
                )
            # tail transpose writes are deferred until after the first
            # pipeline stages dispatch (first NHEAD tiles only need drh)
            def write_drt():
                for g in range(8):
                    nc.sync.dma_start(
                        out=drt_ext[3 * g : 3 * g + 3, :].rearrange(
                            "c (p j) -> p c j", p=128 - PH
                        ),
                        in_=U[PH:128, :].rearrange("p (c j) -> p c j", c=3),
                    )

            def dslice(r0, r1, j0, j1):
                if j1 <= NT * NHEAD:
                    return drh_ext[r0:r1, j0:j1]
                return drt_ext[r0:r1, j0 - NT * NHEAD : j1 - NT * NHEAD]

            def evac_pair(plo, phi, bq, tagp):
                olo = wk.tile([128, NT], FP32R, tag=tagp + "lo")
                ohi = wk.tile([128, NT], FP32R, tag=tagp + "hi")
                nc.scalar.activation(
                    out=olo[:], in_=plo[:], func=AF.Relu,
                    bias=BP[:, 2 * bq : 2 * bq + 1],
                )
                nc.vector.tensor_scalar(
                    out=ohi[:], in0=phi[:], scalar1=BP[:, 2 * bq + 1 : 2 * bq + 2],
                    scalar2=0.0, op0=ALU.add, op1=ALU.max,
                )
                return olo, ohi

            def make_stages(j):
                j0, j1 = NT * j, NT * (j + 1)
                st = {}

                def s0():
                    # merged positional encoding for x (rows 0:60) and d
                    # (rows 64:88); rows 60:64 are zero pad
                    XS = wk.tile([88, NT], FP32, tag="xs")
                    nc.sync.dma_start(out=XS[0:64, :], in_=xrep_ext[:, j0:j1])
                    nc.sync.dma_start(out=XS[64:88, :], in_=dslice(0, 24, j0, j1))
                    Y = wk.tile([88, NT], FP32, tag="y")
                    nc.gpsimd.tensor_scalar(
                        out=Y[:], in0=XS[:], scalar1=CXD[:, 0:1],
                        scalar2=CXD[:, 1:2], op0=ALU.mult, op1=ALU.add,
                    )
                    K1 = wk.tile([88, NT], FP32, tag="k1")
                    nc.gpsimd.tensor_scalar(
                        out=K1[:], in0=Y[:], scalar1=MAGIC, scalar2=None,
                        op0=ALU.add,
                    )
                    K2 = wk.tile([88, NT], FP32, tag="k2")
                    nc.gpsimd.tensor_scalar(
                        out=K2[:], in0=K1[:], scalar1=-MAGIC, scalar2=None,
                        op0=ALU.add,
                    )
                    F = wk.tile([88, NT], FP32, tag="f")
                    nc.gpsimd.tensor_tensor(
                        out=F[:], in0=Y[:], in1=K2[:], op=ALU.subtract
                    )
                    PEXD = pp.tile([91, NT], FP32R, tag="pe")
                    nc.scalar.activation(
                        out=PEXD[0:88, :], in_=F[:], func=AF.Sin, scale=TWO_PI
                    )
                    # raw coords overwrite the pad rows
                    nc.sync.dma_start(
                        out=PEXD[60:63, :],
                        in_=xrep_ext[60:63, j0:j1].bitcast(FP32R),
                    )
                    nc.sync.dma_start(
                        out=PEXD[88:91, :],
                        in_=dslice(0, 3, j0, j1).bitcast(FP32R),
                    )
                    st["pe"] = PEXD

                def s1():
                    plo = ps.tile([128, NT], FP32, tag="pm")
                    phi = ps.tile([128, NT], FP32, tag="pm")
                    o0 = _OFFS[("l1", 0, 0)]
                    o1 = _OFFS[("l1", 0, 1)]
                    PEXD = st["pe"]
                    nc.tensor.matmul(
                        plo[:], WP[0:63, o0 : o0 + 128], PEXD[0:63, :],
                        start=True, stop=True,
                    )
                    nc.tensor.matmul(
                        phi[:], WP[0:63, o1 : o1 + 128], PEXD[0:63, :],
                        start=True, stop=True,
                    )
                    st["h1lo"], st["h1hi"] = evac_pair(plo, phi, 0, "h1")

                def mk_mid(name, src, bq, dst):
                    def s():
                        plo = ps.tile([128, NT], FP32, tag="pm")
                        phi = ps.tile([128, NT], FP32, tag="pm")
                        rhss = (st[src + "lo"][:], st[src + "hi"][:])
                        for k in range(2):
                            o0 = _OFFS[(name, k, 0)]
                            o1 = _OFFS[(name, k, 1)]
                            nc.tensor.matmul(
                                plo[:], WP[0:128, o0 : o0 + 128], rhss[k],
                                start=(k == 0), stop=(k == 1),
                            )
                            nc.tensor.matmul(
                                phi[:], WP[0:128, o1 : o1 + 128], rhss[k],
                                start=(k == 0), stop=(k == 1),
                            )
                        st[dst + "lo"], st[dst + "hi"] = evac_pair(
                            plo, phi, bq, name
                        )
                    return s

                def s5():
                    plo = ps.tile([128, NT], FP32, tag="pm")
                    phi = ps.tile([128, NT], FP32, tag="pm")
                    PEXD = st["pe"]
                    rhss = (st["h4lo"][:], st["h4hi"][:], PEXD[0:63, :])
                    kk = (128, 128, 63)
                    for k in range(3):
                        o0 = _OFFS[("l5", k, 0)]
                        o1 = _OFFS[("l5", k, 1)]
                        nc.tensor.matmul(
                            plo[:], WP[0 : kk[k], o0 : o0 + 128], rhss[k],
                            start=(k == 0), stop=(k == 2),
                        )
                        nc.tensor.matmul(
                            phi[:], WP[0 : kk[k], o1 : o1 + 128], rhss[k],
                            start=(k == 0), stop=(k == 2),
                        )
                    st["h5lo"], st["h5hi"] = evac_pair(plo, phi, 4, "l5")

                def s9():
                    # sigma head
                    psg = ps2.tile([3, NT], FP32, tag="pso")
                    ow0 = _OFFS[("ws", 0, 0)]
                    ow1 = _OFFS[("ws", 1, 0)]
                    nc.tensor.matmul(
                        psg[0:1, :], WP[0:128, ow0 : ow0 + 1], st["h8lo"][:],
                        start=True, stop=False,
                    )
                    nc.tensor.matmul(
                        psg[0:1, :], WP[0:128, ow1 : ow1 + 1], st["h8hi"][:],
                        start=False, stop=True,
                    )
                    SIG = wk.tile([1, NT], FP32, tag="sig")
                    nc.vector.tensor_scalar(
                        out=SIG[:], in0=psg[0:1, :], scalar1=BP[0:1, 19:20],
                        scalar2=0.0, op0=ALU.add, op1=ALU.max,
                    )
                    nc.sync.dma_start(out=sigT_ext[:, j0:j1], in_=SIG[:])
                    # color feature layer
                    plo = ps.tile([128, NT], FP32, tag="pm")
                    phi = ps.tile([128, NT], FP32, tag="pm")
                    rhss = (st["h8lo"][:], st["h8hi"][:])
                    for k in range(2):
                        o0 = _OFFS[("lc", k, 0)]
                        o1 = _OFFS[("lc", k, 1)]
                        nc.tensor.matmul(
                            plo[:], WP[0:128, o0 : o0 + 128], rhss[k],
                            start=(k == 0), stop=(k == 1),
                        )
                        nc.tensor.matmul(
                            phi[:], WP[0:128, o1 : o1 + 128], rhss[k],
                            start=(k == 0), stop=(k == 1),
                        )
                    st["hclo"], st["hchi"] = evac_pair(plo, phi, 8, "lc")

                def s10a():
                    # Wd: K = 256 (hc) + 27 ([enc_d, raw d])
                    phd = ps.tile([128, NT], FP32, tag="pm")
                    PEXD = st["pe"]
                    rhss = (st["hclo"][:], st["hchi"][:], PEXD[64:91, :])
                    base = (0, 0, 64)
                    kk = (128, 128, 27)
                    for k in range(3):
                        od = _OFFS[("wd", k, 0)]
                        nc.tensor.matmul(
                            phd[:],
                            WP[base[k] : base[k] + kk[k], od : od + 128],
                            rhss[k],
                            start=(k == 0), stop=(k == 2),
                        )
                    HD = wk.tile([128, NT], FP32R, tag="hd")
                    nc.vector.tensor_scalar(
                        out=HD[:], in0=phd[:], scalar1=BP[:, 18:19], scalar2=0.0,
                        op0=ALU.add, op1=ALU.max,
                    )
                    st["hd"] = HD

                def s10b():
                    # Wo + sigmoid via 0.5*tanh(0.5 z)+0.5
                    po = ps2.tile([3, NT], FP32, tag="pso")
                    oo = _OFFS[("wo", 0, 0)]
                    nc.tensor.matmul(
                        po[:], WP[0:128, oo : oo + 3], st["hd"][:],
                        start=True, stop=True,
                    )
                    T3 = wk.tile([3, NT], FP32, tag="t3")
                    nc.scalar.activation(
                        out=T3[:], in_=po[:], func=AF.Tanh, bias=BP[0:3, 20:21],
                        scale=0.5,
                    )
                    RGB = wk.tile([3, NT], FP32, tag="rgb")
                    nc.gpsimd.tensor_scalar(
                        out=RGB[:], in0=T3[:], scalar1=0.5, scalar2=0.5,
                        op0=ALU.mult, op1=ALU.add,
                    )
                    nc.sync.dma_start(out=rgbT_ext[:, j0:j1], in_=RGB[:])

                return [
                    s0, s1,
                    mk_mid("l2", "h1", 1, "h2"),
                    mk_mid("l3", "h2", 2, "h3"),
                    mk_mid("l4", "h3", 3, "h4"),
                    s5,
                    mk_mid("l6", "h5", 5, "h6"),
                    mk_mid("l7", "h6", 6, "h7"),
                    mk_mid("l8", "h7", 7, "h8"),
                    s9, s10a, s10b,
                ]

            allst = [make_stages(j) for j in range(NTILES)]
            # prologue: enc for the first pair
            allst[0][0]()
            allst[1][0]()
            write_drt()
            for p in range(0, NTILES, 2):
                for si in range(1, 12):
                    allst[p][si]()
                    allst[p + 1][si]()
                    # enc lookahead: emit next pair's encodings mid-pair,
                    # spread apart to avoid bursting the ACT queue
                    if si == 3 and p + 2 < NTILES:
                        allst[p + 2][0]()
                    if si == 6 and p + 3 < NTILES:
                        allst[p + 3][0]()

    nc.compile()
    _CACHE["nc"] = nc
    return nc


def _enc_consts(L):
    out = np.zeros((6 * L, 2), dtype=np.float32)
    for i in range(L):
        for c in range(6):
            out[6 * i + c, 0] = (2.0**i) * INV_2PI
            out[6 * i + c, 1] = 0.0 if c < 3 else 0.25
    return out


def _pack_weights(inp):
    wp = np.zeros((128, _NCOLS), dtype=np.float32)

    def put(key, block, row0=0):
        off = _OFFS[key]
        K, M = block.shape
        wp[row0 : row0 + K, off : off + M] = block

    W1 = np.asarray(inp["W1"], np.float32)
    l1T = np.concatenate([W1[:, 3:], W1[:, :3]], axis=1).T  # (63, 256)
    put(("l1", 0, 0), l1T[:, 0:128])
    put(("l1", 0, 1), l1T[:, 128:256])
    for name, wname in (
        ("l2", "W2"), ("l3", "W3"), ("l4", "W4"),
        ("l6", "W6"), ("l7", "W7"), ("l8", "W8"), ("lc", "Wc"),
    ):
        WT = np.asarray(inp[wname], np.float32).T  # (256, 256)
        for k in range(2):
            for m in range(2):
                put((name, k, m),
                    WT[128 * k : 128 * (k + 1), 128 * m : 128 * (m + 1)])
    W5 = np.asarray(inp["W5"], np.float32)  # (256, 319); cols [h(256), x(3), enc(60)]
    W5T = np.concatenate([W5[:, :256], W5[:, 259:319], W5[:, 256:259]], axis=1).T
    for k, (a, b) in enumerate(((0, 128), (128, 256), (256, 319))):
        for m in range(2):
            put(("l5", k, m), W5T[a:b, 128 * m : 128 * (m + 1)])
    WsT = np.asarray(inp["Ws"], np.float32).T  # (256, 1)
    put(("ws", 0, 0), WsT[0:128])
    put(("ws", 1, 0), WsT[128:256])
    Wd = np.asarray(inp["Wd"], np.float32)  # (128, 283); cols [hc(256), d(3), enc(24)]
    WdT = np.concatenate([Wd[:, :256], Wd[:, 259:283], Wd[:, 256:259]], axis=1).T
    for k, (a, b) in enumerate(((0, 128), (128, 256), (256, 283))):
        put(("wd", k, 0), WdT[a:b, :], row0=64 if k == 2 else 0)
    put(("wo", 0, 0), np.asarray(inp["Wo"], np.float32).T)  # (128, 3)
    return wp


def _pack_biases(inp):
    bp = np.zeros((128, 21), dtype=np.float32)
    for q, nm in enumerate(("b1", "b2", "b3", "b4", "b5", "b6", "b7", "b8", "bc")):
        b = np.asarray(inp[nm], np.float32)
        bp[:, 2 * q] = b[0:128]
        bp[:, 2 * q + 1] = b[128:256]
    bp[:, 18] = np.asarray(inp["bd"], np.float32)
    bp[0, 19] = float(np.asarray(inp["bs"], np.float32).reshape(-1)[0])
    bp[0:3, 20] = 0.5 * np.asarray(inp["bo"], np.float32)
    return bp


def _enc_consts_merged():
    cxd = np.zeros((88, 2), dtype=np.float32)
    cxd[0:60] = _enc_consts(L_XYZ)
    cxd[64:88] = _enc_consts(L_DIR)
    return cxd


def kernel(**inputs):
    nc = _build()
    x = np.ascontiguousarray(np.asarray(inputs["x_world"], np.float32))
    d = np.ascontiguousarray(np.asarray(inputs["d"], np.float32))
    wp = _pack_weights(inputs)
    bp = _pack_biases(inputs)
    cxd = _enc_consts_merged()

    in_maps = []
    for c in range(NCORES):
        sl = slice(c * BC, (c + 1) * BC)
        xrep = np.zeros((64, BC), dtype=np.float32)
        xrep[0:63] = np.tile(x[sl].T, (21, 1))
        d0 = np.ascontiguousarray(
            d[sl].reshape(128, 256, 3).transpose(0, 2, 1).reshape(128, 768)
        )
        in_maps.append({"xrep": xrep, "d0": d0, "wp": wp, "bp": bp, "cxd": cxd})
    _CACHE["in_maps"] = in_maps
    res = run_bass_kernel_spmd(nc, in_maps, list(range(NCORES)))
    rgb = np.concatenate(
        [res.results[c]["rgbT"].T for c in range(NCORES)], axis=0
    ).astype(np.float32)
    sigma = np.concatenate(
        [res.results[c]["sigT"].T for c in range(NCORES)], axis=0
    ).astype(np.float32)
    return rgb, sigma
